# revision 2
# baseline (speedup 1.0000x reference)
"""DGCNN part-segmentation forward pass for nn_DC_Net_56856777064808 on 8 trn2 NeuronCores.

Sharding (per the data-parallel hint): 8 cores = 2 batches x 4 query-chunks of
1024 points. Each core holds the full per-cloud coordinates/features (small)
and computes kNN + gather + edge-convs for its 1024 query points. Feature maps
produced per-chunk (h1, h2) are exchanged with jax.lax.all_gather within each
4-core batch group; the transform-net global max uses lax.pmax. Head convs and
softmax are per-point (chunk-local). Output chunks are reassembled on host.
"""
import os

os.environ.setdefault(
    "NEURON_CC_FLAGS",
    "--auto-cast=none",  # keep fp32 matmuls fp32: kNN neighbor sets must match fp32 reference
)

import numpy as np

K = 20
RSQ = 1.0 / np.sqrt(1.0 + 1e-5)
B, C0, N = 2, 3, 4096
NCORES = 8
GROUPS = [[0, 1, 2, 3], [4, 5, 6, 7]]
NQ = N // 4  # 1024 queries per core


def _build(jnp, jax):
    def lrelu(x):
        return jnp.where(x >= 0, x, 0.2 * x)

    def cbl(x, w, bn):
        # x: (C, ...) unbatched; 1x1 conv + eval BN + LeakyReLU
        y = jnp.einsum("oc,c...->o...", w, x)
        sh = (-1,) + (1,) * (y.ndim - 1)
        return lrelu(y * (bn[0] * RSQ).reshape(sh) + bn[1].reshape(sh))

    def knn_chunk(xq, xf):
        # xq: (C, NQ) queries, xf: (C, N) full cloud -> idx (NQ, K)
        xxq = jnp.sum(xq * xq, axis=0)
        xxf = jnp.sum(xf * xf, axis=0)
        inner = jnp.einsum("cq,cn->qn", xq, xf)
        negd = 2.0 * inner - xxq[:, None] - xxf[None, :]
        return jax.lax.top_k(negd, K)[1]

    def graph_feature_chunk(xq, xf):
        # -> (2C, NQ, K) with [neighbor - center, center]
        idx = knn_chunk(xq, xf)
        nbr = xf.T[idx]                       # (NQ, K, C)
        ctr = jnp.broadcast_to(xq.T[:, None, :], nbr.shape)
        f = jnp.concatenate([nbr - ctr, ctr], axis=-1)
        return jnp.transpose(f, (2, 0, 1))

    def step(xf, xq, p):
        # xf: (3, N) full cloud of this core's batch; xq: (3, NQ) its query slice
        # p: dict of weights (replicated)
        # ---- Transform_Net ----
        f0 = graph_feature_chunk(xq, xf)                       # (6, NQ, K)
        h = cbl(cbl(cbl(f0, p["tw1"], p["tb1"]), p["tw2"], p["tb2"]), p["tw3"], p["tb3"])
        h = h.max(axis=-1)                                     # (128, NQ)
        h = cbl(h, p["tw4"], p["tb4"]).max(axis=-1)            # (1024,) local max
        h = jax.lax.pmax(h, "i", axis_index_groups=GROUPS)     # global over N
        h = cbl(cbl(h, p["tl1"], p["tb5"]), p["tl2"], p["tb6"])
        t = (h @ p["ttw"].T + p["ttb"]).reshape(3, 3)
        xf2 = jnp.einsum("cn,cd->dn", xf, t)                   # transformed cloud
        xq2 = jnp.einsum("cn,cd->dn", xq, t)

        def edge_block(fq, ff, w1, b1, w2, b2, w3, b3):
            f = graph_feature_chunk(fq, ff)                    # (2C, NQ, K)
            h = cbl(cbl(cbl(f, w1, b1), w2, b2), w3, b3).max(axis=-1)
            return h                                           # (64, NQ)

        def allgather_pts(hc):
            # (C, NQ) chunk -> (C, N) full via in-group all_gather
            g = jax.lax.all_gather(hc, "i", axis_index_groups=GROUPS)  # (4, C, NQ)
            return jnp.transpose(g, (1, 0, 2)).reshape(hc.shape[0], -1)

        # ---- EdgeConv 1 ----  (x3 = [h1; h1])
        h1 = edge_block(xq2, xf2, p["w1"], p["b1"], p["w2"], p["b2"], p["w3"], p["b3"])
        h1f = allgather_pts(h1)
        x3q = jnp.concatenate([h1, h1], axis=0)                # (128, NQ)
        x3f = jnp.concatenate([h1f, h1f], axis=0)
        # ---- EdgeConv 2 ----
        h2 = edge_block(x3q, x3f, p["w4"], p["b4"], p["w5"], p["b5"], p["w6"], p["b6"])
        h2f = allgather_pts(h2)
        x4q = jnp.concatenate([h2, h2], axis=0)
        x4f = jnp.concatenate([h2f, h2f], axis=0)
        # ---- EdgeConv 3 ----
        x5q = edge_block(x4q, x4f, p["w7"], p["b7"], p["w8"], p["b8"], p["w9"], p["b9"])
        # ---- head (per-point) ----
        cat = jnp.concatenate([x3q, x4q, x5q], axis=0)         # (320, NQ)
        g = cbl(cat, p["w10"], p["b10"])                       # (1024, NQ)
        hh = jnp.concatenate([g, x3q, x4q, x5q], axis=0)       # (1344, NQ)
        hh = cbl(cbl(cbl(hh, p["w11"], p["b11"]), p["w12"], p["b12"]), p["w13"], p["b13"])
        logits = jnp.einsum("oc,cn->on", p["w14"], hh)         # (17, NQ)
        return jax.nn.softmax(logits.T, axis=-1)               # (NQ, 17)

    return step


def _run_sharded(inputs, jax, jnp, devices):
    x = np.asarray(inputs["x"])[:, 0]  # (2, 3, 4096)
    pnames = [k for k in inputs.keys() if k != "x"]
    params = {k: jnp.asarray(np.asarray(inputs[k])) for k in pnames}

    xf = np.stack([x[c // 4] for c in range(NCORES)])                       # (8, 3, N)
    xq = np.stack([x[c // 4][:, (c % 4) * NQ:(c % 4 + 1) * NQ] for c in range(NCORES)])

    step = _build(jnp, jax)
    f = jax.pmap(step, axis_name="i", in_axes=(0, 0, None), devices=devices)
    out = np.asarray(f(xf, xq, params))                                      # (8, NQ, 17)
    full = np.zeros((B, N, 17), dtype=np.float32)
    for c in range(NCORES):
        full[c // 4, (c % 4) * NQ:(c % 4 + 1) * NQ] = out[c]
    return full


def kernel(**inputs) -> np.ndarray:
    import jax
    import jax.numpy as jnp

    try:
        devices = [d for d in jax.devices() if d.platform != "cpu"][:NCORES]
        if len(devices) == NCORES:
            return _run_sharded(inputs, jax, jnp, devices)
    except Exception as e:  # noqa: BLE001 - fall back to host execution on any device failure
        import traceback
        traceback.print_exc()
        print(f"[kernel] device path failed ({type(e).__name__}: {e}); falling back to CPU")

    return _run_cpu(inputs, jax, jnp)


def _run_cpu(inputs, jax, jnp):
    # Single-device CPU fallback: same math, unsharded.
    with jax.default_device(jax.devices("cpu")[0]):
        x = jnp.asarray(np.asarray(inputs["x"]))[:, 0]
        params = {k: jnp.asarray(np.asarray(v)) for k, v in inputs.items() if k != "x"}
        step = _build(jnp, jax)

        # emulate the sharded program without collectives: full N as one "chunk"
        def pmax_id(v, *_a, **_k):
            return v

        orig_pmax, orig_ag = jax.lax.pmax, jax.lax.all_gather
        jax.lax.pmax = pmax_id
        jax.lax.all_gather = lambda v, *_a, **_k: v[None]
        try:
            outs = []
            for b in range(B):
                outs.append(np.asarray(step(x[b], x[b], params)))
        finally:
            jax.lax.pmax, jax.lax.all_gather = orig_pmax, orig_ag
        return np.stack(outs).astype(np.float32)


# revision 5
# speedup vs baseline: 7.6372x; 7.6372x over previous
"""DGCNN part-segmentation forward pass for nn_DC_Net_56856777064808 on 8 trn2 NeuronCores.

Sharding (per the data-parallel hint): 8 cores = 2 batches x 4 query-chunks of
1024 points. Each core holds the full per-cloud coordinates/features (small)
and computes kNN + gather + edge-convs for its 1024 query points. Feature maps
produced per-chunk (h1, h2) are exchanged with jax.lax.all_gather within each
4-core batch group; the transform-net global max uses lax.pmax. Head convs and
softmax are per-point (chunk-local). Output chunks are reassembled on host.
"""
import os

os.environ.setdefault(
    "NEURON_CC_FLAGS",
    "--auto-cast=none",  # keep fp32 matmuls fp32: kNN neighbor sets must match fp32 reference
)

import numpy as np

K = 20
RSQ = 1.0 / np.sqrt(1.0 + 1e-5)
B, C0, N = 2, 3, 4096
NCORES = 8
GROUPS = [[0, 1, 2, 3], [4, 5, 6, 7]]
NQ = N // 4  # 1024 queries per core


def _build(jnp, jax):
    def lrelu(x):
        return jnp.where(x >= 0, x, 0.2 * x)

    def cbl(x, w, bn):
        # x: (C, ...) unbatched; 1x1 conv + eval BN + LeakyReLU
        y = jnp.einsum("oc,c...->o...", w, x)
        sh = (-1,) + (1,) * (y.ndim - 1)
        return lrelu(y * (bn[0] * RSQ).reshape(sh) + bn[1].reshape(sh))

    def knn_chunk(xq, xf):
        # xq: (C, NQ) queries, xf: (C, N) full cloud -> idx (NQ, K)
        xxq = jnp.sum(xq * xq, axis=0)
        xxf = jnp.sum(xf * xf, axis=0)
        inner = jnp.einsum("cq,cn->qn", xq, xf)
        negd = 2.0 * inner - xxq[:, None] - xxf[None, :]
        return jax.lax.top_k(negd, K)[1]

    def graph_feature_chunk(xq, xf):
        # -> (2C, NQ, K) with [neighbor - center, center]
        idx = knn_chunk(xq, xf)
        nbr = xf.T[idx]                       # (NQ, K, C)
        ctr = jnp.broadcast_to(xq.T[:, None, :], nbr.shape)
        f = jnp.concatenate([nbr - ctr, ctr], axis=-1)
        return jnp.transpose(f, (2, 0, 1))

    def prep_uv(w, bn, fold_dup):
        # conv over [nbr-ctr; ctr] == Wa@nbr + (Wb-Wa)@ctr; BN scale folded in.
        # fold_dup: input features are [h; h] duplicated -> fold weight halves.
        g = (bn[0] * RSQ)[:, None]
        C = w.shape[1] // 2
        Wa, Wv = w[:, :C], w[:, C:] - w[:, :C]
        if fold_dup:
            Wa = Wa[:, : C // 2] + Wa[:, C // 2:]
            Wv = Wv[:, : C // 2] + Wv[:, C // 2:]
        return g * Wa, g * Wv, bn[1][:, None]

    def edge_block_uv(fq, ff, wb1, w2, b2, w3, b3):
        # first conv applied per-point before the gather (u/v trick)
        Wa, Wv, bb = wb1
        idx = knn_chunk(fq, ff)
        u = Wa @ ff                                            # (64, Nf)
        v = Wv @ fq + bb                                       # (64, NQ)
        f1 = lrelu(jnp.transpose(u.T[idx], (2, 0, 1)) + v[:, :, None])
        return cbl(cbl(f1, w2, b2), w3, b3).max(axis=-1)       # (64, NQ)

    def step(xf, xq, p):
        # xf: (3, N) full cloud of this core's batch; xq: (3, NQ) its query slice
        # p: dict of weights (replicated)
        # ---- Transform_Net ----
        h = edge_block_uv(xq, xf, prep_uv(p["tw1"], p["tb1"], False),
                          p["tw2"], p["tb2"], p["tw3"], p["tb3"])
        h = cbl(h, p["tw4"], p["tb4"]).max(axis=-1)            # (1024,) local max
        h = jax.lax.pmax(h, "i", axis_index_groups=GROUPS)     # global over N
        h = cbl(cbl(h, p["tl1"], p["tb5"]), p["tl2"], p["tb6"])
        t = (h @ p["ttw"].T + p["ttb"]).reshape(3, 3)
        xf2 = jnp.einsum("cn,cd->dn", xf, t)                   # transformed cloud
        xq2 = jnp.einsum("cn,cd->dn", xq, t)

        def allgather_pts(hc):
            # (C, NQ) chunk -> (C, N) full via in-group all_gather
            g = jax.lax.all_gather(hc, "i", axis_index_groups=GROUPS)  # (4, C, NQ)
            return jnp.transpose(g, (1, 0, 2)).reshape(hc.shape[0], -1)

        # ---- EdgeConv 1 ----  (x3 = [h1; h1])
        h1 = edge_block_uv(xq2, xf2, prep_uv(p["w1"], p["b1"], False),
                           p["w2"], p["b2"], p["w3"], p["b3"])
        h1f = allgather_pts(h1)
        # ---- EdgeConv 2 ----  kNN on x3=[h;h] == kNN on h (scores scale by 2)
        h2 = edge_block_uv(h1, h1f, prep_uv(p["w4"], p["b4"], True),
                           p["w5"], p["b5"], p["w6"], p["b6"])
        h2f = allgather_pts(h2)
        # ---- EdgeConv 3 ----
        x5q = edge_block_uv(h2, h2f, prep_uv(p["w7"], p["b7"], True),
                            p["w8"], p["b8"], p["w9"], p["b9"])
        # ---- head (per-point); fold duplicated [h;h] channels into weights ----
        w10 = p["w10"]
        w10f = jnp.concatenate([w10[:, :64] + w10[:, 64:128],
                                w10[:, 128:192] + w10[:, 192:256],
                                w10[:, 256:320]], axis=1)       # (1024, 192)
        cat3 = jnp.concatenate([h1, h2, x5q], axis=0)           # (192, NQ)
        g = cbl(cat3, w10f, p["b10"])                           # (1024, NQ)
        w11 = p["w11"]
        w11f = jnp.concatenate([w11[:, :1024],
                                w11[:, 1024:1088] + w11[:, 1088:1152],
                                w11[:, 1152:1216] + w11[:, 1216:1280],
                                w11[:, 1280:1344]], axis=1)     # (256, 1216)
        hh = jnp.concatenate([g, cat3], axis=0)                 # (1216, NQ)
        hh = cbl(cbl(cbl(hh, w11f, p["b11"]), p["w12"], p["b12"]), p["w13"], p["b13"])
        logits = jnp.einsum("oc,cn->on", p["w14"], hh)          # (17, NQ)
        return jax.nn.softmax(logits.T, axis=-1)                # (NQ, 17)

    return step


_CACHE = {}


def _run_sharded(inputs, jax, jnp, devices):
    x = np.asarray(inputs["x"])[:, 0]  # (2, 3, 4096)

    xf = np.stack([x[c // 4] for c in range(NCORES)])                       # (8, 3, N)
    xq = np.stack([x[c // 4][:, (c % 4) * NQ:(c % 4 + 1) * NQ] for c in range(NCORES)])

    if "f" not in _CACHE:
        step = _build(jnp, jax)
        _CACHE["f"] = jax.pmap(step, axis_name="i", in_axes=(0, 0, 0), devices=devices)
        params = {k: np.asarray(v) for k, v in inputs.items() if k != "x"}
        _CACHE["params"] = jax.device_put_replicated(params, devices)
    out = np.asarray(_CACHE["f"](xf, xq, _CACHE["params"]))                  # (8, NQ, 17)
    full = np.zeros((B, N, 17), dtype=np.float32)
    for c in range(NCORES):
        full[c // 4, (c % 4) * NQ:(c % 4 + 1) * NQ] = out[c]
    return full


def kernel(**inputs) -> np.ndarray:
    import jax
    import jax.numpy as jnp

    try:
        devices = [d for d in jax.devices() if d.platform != "cpu"][:NCORES]
        if len(devices) == NCORES:
            return _run_sharded(inputs, jax, jnp, devices)
    except Exception as e:  # noqa: BLE001 - fall back to host execution on any device failure
        import traceback
        traceback.print_exc()
        print(f"[kernel] device path failed ({type(e).__name__}: {e}); falling back to CPU")

    return _run_cpu(inputs, jax, jnp)


def _run_cpu(inputs, jax, jnp):
    # Single-device CPU fallback: same math, unsharded.
    with jax.default_device(jax.devices("cpu")[0]):
        x = jnp.asarray(np.asarray(inputs["x"]))[:, 0]
        params = {k: jnp.asarray(np.asarray(v)) for k, v in inputs.items() if k != "x"}
        step = _build(jnp, jax)

        # emulate the sharded program without collectives: full N as one "chunk"
        def pmax_id(v, *_a, **_k):
            return v

        orig_pmax, orig_ag = jax.lax.pmax, jax.lax.all_gather
        jax.lax.pmax = pmax_id
        jax.lax.all_gather = lambda v, *_a, **_k: v[None]
        try:
            outs = []
            for b in range(B):
                outs.append(np.asarray(step(x[b], x[b], params)))
        finally:
            jax.lax.pmax, jax.lax.all_gather = orig_pmax, orig_ag
        return np.stack(outs).astype(np.float32)
